# revision 1
# baseline (speedup 1.0000x reference)
"""Trainium2 Bass kernel for LocalAveragePoolingSegmenter (segment mean-pool).

Strategy: pure data-parallel over batch (2 batches per core on 8 cores).
Per batch, instead of the O(Tt*Ta*D) masked einsum, compute per-128-frame
local cumsums of the audio with triangular fp32 matmuls, store them to a
DRAM table, and reconstruct each token's segment sum with two indirect-DMA
row gathers plus a tiny signed-one-hot matmul against a 33-row block-offset
table. Host precomputes all index/one-hot/reciprocal tensors from
asr_alignment / text_token_len (tiny int tensors); all heavy data stays on
device.
"""

import numpy as np

B, TA, TT, D = 16, 4096, 1024, 512
NCORES = 8
BPC = B // NCORES       # batches per core
NBLK = TA // 128        # 32 time blocks per batch
NG = 4                  # audio DMA groups (8 blocks = 1024 rows each)
BLKG = NBLK // NG       # 8 blocks per group
NTOK = TT // 128        # 8 token tiles per batch
ZROW = TA               # index of the all-zero row in the P table
POFF = NBLK + 1         # 33 rows in block-offset table

_CACHE = {}


def _build_program():
    import concourse.bass as bass
    import concourse.tile as tile
    from concourse import bacc, mybir

    nc = bacc.Bacc("TRN2", target_bir_lowering=False, debug=False,
                   enable_asserts=False, num_devices=NCORES)

    f32, i32, f32d = mybir.dt.float32, mybir.dt.int32, mybir.dt.float32
    audio_in = nc.dram_tensor("audio", [BPC, TA, D], f32, kind="ExternalInput").ap()
    pidx_in = nc.dram_tensor("pidx", [BPC, 128, 2 * NTOK], i32, kind="ExternalInput").ap()
    bt_in = nc.dram_tensor("bt", [BPC, POFF, TT], f32, kind="ExternalInput").ap()
    recip_in = nc.dram_tensor("recip", [BPC, 128, NTOK], f32, kind="ExternalInput").ap()
    lridx_in = nc.dram_tensor("lridx", [NBLK, 1], i32, kind="ExternalInput").ap()
    seg_out = nc.dram_tensor("seg", [BPC, TT, D], f32, kind="ExternalOutput").ap()

    with tile.TileContext(nc) as tc:
        with (
            tc.tile_pool(name="const", bufs=1) as cpool,
            tc.tile_pool(name="xg", bufs=3) as xpool,
            tc.tile_pool(name="pg", bufs=3) as pgpool,
            tc.tile_pool(name="small", bufs=2) as spool,
            tc.tile_pool(name="gath", bufs=3) as gpool,
            tc.tile_pool(name="outp", bufs=3) as opool,
            tc.tile_pool(name="ps", bufs=4, space="PSUM") as pspool,
            tc.tile_pool(name="pstok", bufs=2, space="PSUM") as pstokpool,
            tc.tile_pool(name="psoff", bufs=2, space="PSUM") as psoffpool,
            tc.tile_pool(name="pdram", bufs=2, space="DRAM") as dpool,
        ):
            # constants ---------------------------------------------------
            # ut[k, m] = 1.0 if k <= m else 0 (inclusive-cumsum weights)
            ut = cpool.tile([128, 128], f32)
            nc.gpsimd.memset(ut[:], 0.0)
            nc.gpsimd.affine_select(
                out=ut[:], in_=ut[:], compare_op=mybir.AluOpType.is_gt,
                fill=1.0, base=0, pattern=[[-1, 128]], channel_multiplier=1,
            )
            # stut[j, m] = 1.0 if j < m else 0 (strict cumsum for offsets)
            stut = cpool.tile([NBLK, POFF], f32)
            nc.gpsimd.memset(stut[:], 0.0)
            nc.gpsimd.affine_select(
                out=stut[:], in_=stut[:], compare_op=mybir.AluOpType.is_ge,
                fill=1.0, base=0, pattern=[[-1, POFF]], channel_multiplier=1,
            )
            zrow = cpool.tile([1, D], f32)
            nc.vector.memset(zrow[:], 0.0)
            lridx = cpool.tile([NBLK, 1], i32)
            nc.sync.dma_start(lridx[:], lridx_in[:])

            for b in range(BPC):
                # ---- phase A: local cumsums -> P table in DRAM ----------
                ptab = dpool.tile([TA + 1, D], f32)
                for g in range(NG):
                    xg = xpool.tile([128, BLKG * D], f32, tag="xg")
                    nc.sync.dma_start(
                        xg[:].rearrange("p (k d) -> p k d", k=BLKG),
                        audio_in[b, 1024 * g : 1024 * (g + 1), :]
                        .rearrange("(k p) d -> p k d", p=128),
                    )
                    pgt = pgpool.tile([128, BLKG * D], f32, tag="pg")
                    for k8 in range(BLKG):
                        psc = pspool.tile([128, D], f32, tag="psc")
                        nc.tensor.matmul(
                            out=psc[:], lhsT=ut[:],
                            rhs=xg[:, bass.ts(k8, D)],
                            start=True, stop=True,
                        )
                        eng = nc.vector if (k8 % 2 == 0) else nc.scalar
                        if eng is nc.vector:
                            nc.vector.tensor_copy(pgt[:, bass.ts(k8, D)], psc[:])
                        else:
                            nc.scalar.copy(pgt[:, bass.ts(k8, D)], psc[:])
                    nc.sync.dma_start(
                        ptab[1024 * g : 1024 * (g + 1), :]
                        .rearrange("(k p) d -> p k d", p=128),
                        pgt[:].rearrange("p (k d) -> p k d", k=BLKG),
                    )
                nc.sync.dma_start(ptab[ZROW : ZROW + 1, :], zrow[:])

                # ---- block-offset table off[33, D] ----------------------
                lrows = spool.tile([NBLK, D], f32, tag="lrows")
                nc.gpsimd.indirect_dma_start(
                    out=lrows[:], out_offset=None, in_=ptab[:],
                    in_offset=bass.IndirectOffsetOnAxis(ap=lridx[:, :1], axis=0),
                )
                psoff = psoffpool.tile([POFF, D], f32, tag="psoff")
                nc.tensor.matmul(out=psoff[:], lhsT=stut[:], rhs=lrows[:],
                                 start=True, stop=True)
                off_sb = spool.tile([POFF, D], f32, tag="off")
                nc.vector.tensor_copy(off_sb[:], psoff[:])

                # ---- per-batch small inputs -----------------------------
                idx_sb = spool.tile([128, 2 * NTOK], i32, tag="idx")
                nc.sync.dma_start(idx_sb[:], pidx_in[b])
                bt_sb = spool.tile([POFF, TT], f32, tag="bt")
                nc.sync.dma_start(bt_sb[:], bt_in[b])
                rc_sb = spool.tile([128, NTOK], f32, tag="rc")
                nc.sync.dma_start(rc_sb[:], recip_in[b])

                # ---- phase B: gather + combine per token tile -----------
                for j in range(NTOK):
                    ge = gpool.tile([128, D], f32, tag="ge")
                    nc.gpsimd.indirect_dma_start(
                        out=ge[:], out_offset=None, in_=ptab[:],
                        in_offset=bass.IndirectOffsetOnAxis(
                            ap=idx_sb[:, 2 * j : 2 * j + 1], axis=0),
                    )
                    gs = gpool.tile([128, D], f32, tag="gs")
                    nc.gpsimd.indirect_dma_start(
                        out=gs[:], out_offset=None, in_=ptab[:],
                        in_offset=bass.IndirectOffsetOnAxis(
                            ap=idx_sb[:, 2 * j + 1 : 2 * j + 2], axis=0),
                    )
                    pstok = pstokpool.tile([128, D], f32, tag="pstok")
                    nc.tensor.matmul(out=pstok[:],
                                     lhsT=bt_sb[:, bass.ts(j, 128)],
                                     rhs=off_sb[:], start=True, stop=True)
                    d1 = gpool.tile([128, D], f32, tag="d1")
                    nc.vector.tensor_sub(d1[:], ge[:], gs[:])
                    nc.vector.tensor_add(d1[:], d1[:], pstok[:])
                    ot = opool.tile([128, D], f32, tag="ot")
                    nc.scalar.mul(ot[:], d1[:], rc_sb[:, j : j + 1])
                    nc.sync.dma_start(seg_out[b, bass.ts(j, 128), :], ot[:])

    nc.compile()
    return nc


def _get_program():
    if "nc" not in _CACHE:
        _CACHE["nc"] = _build_program()
    return _CACHE["nc"]


def _host_prep(asr_alignment, text_token_len):
    a = np.asarray(asr_alignment).astype(np.int64)
    s, e = a[..., 0], a[..., 1]
    tlen = np.asarray(text_token_len).astype(np.int64)
    tmask = np.arange(TT)[None, :] < tlen[:, None]

    pe = np.where(((e + 1) % 128) != 0, e, ZROW)
    ps = np.where((s % 128) != 0, s - 1, ZROW)
    pe = np.where(tmask, pe, ZROW)
    ps = np.where(tmask, ps, ZROW)
    be = np.where(tmask, (e + 1) >> 7, 0)
    bs = np.where(tmask, s >> 7, 0)

    cnt = (e - s + 1).astype(np.float64)
    recip = np.where(tmask, 1.0 / cnt, 0.0).astype(np.float32)

    pidx = np.empty((B, 128, 2 * NTOK), np.int32)
    pidx[:, :, 0::2] = pe.reshape(B, NTOK, 128).transpose(0, 2, 1)
    pidx[:, :, 1::2] = ps.reshape(B, NTOK, 128).transpose(0, 2, 1)

    ks = np.arange(POFF)[None, :, None]
    bt = ((ks == be[:, None, :]).astype(np.float32)
          - (ks == bs[:, None, :]).astype(np.float32))

    recip_dev = recip.reshape(B, NTOK, 128).transpose(0, 2, 1)
    recip_dev = np.ascontiguousarray(recip_dev, np.float32)
    return pidx, np.ascontiguousarray(bt), recip_dev


def _run(inputs_by_core, trace=False, **kw):
    from concourse.bass_utils import run_bass_kernel_spmd
    nc = _get_program()
    return run_bass_kernel_spmd(nc, inputs_by_core,
                                core_ids=list(range(NCORES)), trace=trace, **kw)


def _make_in_maps(audio_feats, asr_alignment, text_token_len):
    audio = np.ascontiguousarray(np.asarray(audio_feats), np.float32)
    pidx, bt, recip = _host_prep(asr_alignment, text_token_len)
    lridx = (np.arange(NBLK, dtype=np.int32) * 128 + 127).reshape(NBLK, 1)
    in_maps = []
    for c in range(NCORES):
        sl = slice(BPC * c, BPC * (c + 1))
        in_maps.append({
            "audio": audio[sl],
            "pidx": pidx[sl],
            "bt": bt[sl],
            "recip": recip[sl],
            "lridx": lridx,
        })
    return in_maps


def kernel(audio_feats, audio_feats_len, text_token_for_audio,
           text_token_embed_for_audio, text_token_len, asr_alignment,
           _trace=False, **_kw):
    in_maps = _make_in_maps(audio_feats, asr_alignment, text_token_len)
    res = _run(in_maps, trace=_trace, **_kw)
    seg = np.concatenate([res.results[c]["seg"] for c in range(NCORES)], axis=0)
    out_len = np.asarray(text_token_len).astype(np.int32, copy=False)
    if _trace:
        return (seg, out_len), res
    return seg, out_len


# revision 9
# speedup vs baseline: 1.0507x; 1.0507x over previous
"""Trainium2 Bass kernel for LocalAveragePoolingSegmenter (segment mean-pool).

Strategy: pure data-parallel over batch (2 batches per core on 8 cores).
Per batch, instead of the O(Tt*Ta*D) masked einsum, compute per-128-frame
local cumsums of the audio with triangular fp32 matmuls, store them to a
DRAM table, and reconstruct each token's segment sum with two indirect-DMA
row gathers plus a tiny signed-one-hot matmul against a 33-row block-offset
table. Host precomputes all index/one-hot/reciprocal tensors from
asr_alignment / text_token_len (tiny int tensors); all heavy data stays on
device.
"""

import numpy as np

B, TA, TT, D = 16, 4096, 1024, 512
NCORES = 8
BPC = B // NCORES       # batches per core
NBLK = TA // 128        # 32 time blocks per batch
NG = 4                  # audio DMA groups (8 blocks = 1024 rows each)
BLKG = NBLK // NG       # 8 blocks per group
NTOK = TT // 128        # 8 token tiles per batch
ZROW = TA               # index of the all-zero row in the P table
POFF = NBLK + 1         # 33 rows in block-offset table

_CACHE = {}


def _build_program():
    import concourse.bass as bass
    import concourse.tile as tile
    from concourse import bacc, mybir

    nc = bacc.Bacc("TRN2", target_bir_lowering=False, debug=False,
                   enable_asserts=False, num_devices=NCORES)

    f32, i32, bf16 = mybir.dt.float32, mybir.dt.int32, mybir.dt.bfloat16
    ahi_in = nc.dram_tensor("audio_hi", [BPC, TA, D], bf16, kind="ExternalInput").ap()
    alo_in = nc.dram_tensor("audio_lo", [BPC, TA, D], bf16, kind="ExternalInput").ap()
    pidx_in = nc.dram_tensor("pidx", [BPC, 128, 2 * NTOK], i32, kind="ExternalInput").ap()
    bt_in = nc.dram_tensor("bt", [BPC, POFF, TT], bf16, kind="ExternalInput").ap()
    recip_in = nc.dram_tensor("recip", [BPC, 128, NTOK], f32, kind="ExternalInput").ap()
    lridx_in = nc.dram_tensor("lridx", [NBLK, 1], i32, kind="ExternalInput").ap()
    seg_out = nc.dram_tensor("seg", [BPC, TT, D], f32, kind="ExternalOutput").ap()

    with tile.TileContext(nc) as tc:
        with (
            tc.tile_pool(name="const", bufs=1) as cpool,
            tc.tile_pool(name="xg", bufs=3) as xpool,
            tc.tile_pool(name="pg", bufs=3) as pgpool,
            tc.tile_pool(name="small", bufs=2) as spool,
            tc.tile_pool(name="gath", bufs=3) as gpool,
            tc.tile_pool(name="outp", bufs=3) as opool,
            tc.tile_pool(name="ps", bufs=4, space="PSUM") as pspool,
            tc.tile_pool(name="pstok", bufs=2, space="PSUM") as pstokpool,
            tc.tile_pool(name="psoff", bufs=2, space="PSUM") as psoffpool,
            tc.tile_pool(name="pdram", bufs=2, space="DRAM") as dpool,
        ):
            # constants ---------------------------------------------------
            # ut[k, m] = 1.0 if k <= m else 0 (inclusive-cumsum weights)
            utf = cpool.tile([128, 128], f32)
            nc.gpsimd.memset(utf[:], 0.0)
            nc.gpsimd.affine_select(
                out=utf[:], in_=utf[:], compare_op=mybir.AluOpType.is_gt,
                fill=1.0, base=0, pattern=[[-1, 128]], channel_multiplier=1,
            )
            ut = cpool.tile([128, 128], bf16)
            nc.vector.tensor_copy(ut[:], utf[:])
            # stut[j, m] = 1.0 if j < m else 0 (strict cumsum for offsets)
            stut = cpool.tile([NBLK, POFF], f32)
            nc.gpsimd.memset(stut[:], 0.0)
            nc.gpsimd.affine_select(
                out=stut[:], in_=stut[:], compare_op=mybir.AluOpType.is_ge,
                fill=1.0, base=0, pattern=[[-1, POFF]], channel_multiplier=1,
            )
            zrow = cpool.tile([1, D], f32)
            nc.vector.memset(zrow[:], 0.0)
            lridx = cpool.tile([NBLK, 1], i32)
            nc.sync.dma_start(lridx[:], lridx_in[:])

            for b in range(BPC):
                # ---- phase A: local cumsums -> P table in DRAM ----------
                ptab = dpool.tile([TA + 1, D], f32)
                for g in range(NG):
                    xh = xpool.tile([128, BLKG * D], bf16, tag="xh")
                    nc.sync.dma_start(
                        xh[:].rearrange("p (k d) -> p k d", k=BLKG),
                        ahi_in[b, 1024 * g : 1024 * (g + 1), :]
                        .rearrange("(k p) d -> p k d", p=128),
                    )
                    xl = xpool.tile([128, BLKG * D], bf16, tag="xl")
                    nc.sync.dma_start(
                        xl[:].rearrange("p (k d) -> p k d", k=BLKG),
                        alo_in[b, 1024 * g : 1024 * (g + 1), :]
                        .rearrange("(k p) d -> p k d", p=128),
                    )
                    pgt = pgpool.tile([128, BLKG * D], f32, tag="pg")
                    for k8 in range(BLKG):
                        psc = pspool.tile([128, D], f32, tag="psc")
                        nc.tensor.matmul(
                            out=psc[:], lhsT=ut[:],
                            rhs=xh[:, bass.ts(k8, D)],
                            start=True, stop=False,
                        )
                        nc.tensor.matmul(
                            out=psc[:], lhsT=ut[:],
                            rhs=xl[:, bass.ts(k8, D)],
                            start=False, stop=True,
                        )
                        eng = nc.vector if (k8 % 2 == 0) else nc.scalar
                        if eng is nc.vector:
                            nc.vector.tensor_copy(pgt[:, bass.ts(k8, D)], psc[:])
                        else:
                            nc.scalar.copy(pgt[:, bass.ts(k8, D)], psc[:])
                    nc.sync.dma_start(
                        ptab[1024 * g : 1024 * (g + 1), :]
                        .rearrange("(k p) d -> p k d", p=128),
                        pgt[:].rearrange("p (k d) -> p k d", k=BLKG),
                    )
                nc.sync.dma_start(ptab[ZROW : ZROW + 1, :], zrow[:])

                # ---- block-offset table off[33, D] ----------------------
                lrows = spool.tile([NBLK, D], f32, tag="lrows")
                nc.gpsimd.indirect_dma_start(
                    out=lrows[:], out_offset=None, in_=ptab[:],
                    in_offset=bass.IndirectOffsetOnAxis(ap=lridx[:, :1], axis=0),
                )
                psoff = psoffpool.tile([POFF, D], f32, tag="psoff")
                nc.tensor.matmul(out=psoff[:], lhsT=stut[:], rhs=lrows[:],
                                 start=True, stop=True)
                off_sb = spool.tile([POFF, D], f32, tag="off")
                nc.vector.tensor_copy(off_sb[:], psoff[:])
                # split offsets into bf16 hi+lo for fast token matmuls
                off_hi = spool.tile([POFF, D], bf16, tag="offh")
                nc.vector.tensor_copy(off_hi[:], off_sb[:])
                off_hf = spool.tile([POFF, D], f32, tag="offhf")
                nc.vector.tensor_copy(off_hf[:], off_hi[:])
                off_lo = spool.tile([POFF, D], bf16, tag="offl")
                nc.vector.tensor_sub(off_lo[:], off_sb[:], off_hf[:])

                # ---- per-batch small inputs -----------------------------
                idx_sb = spool.tile([128, 2 * NTOK], i32, tag="idx")
                nc.sync.dma_start(idx_sb[:], pidx_in[b])
                bt_sb = spool.tile([POFF, TT], bf16, tag="bt")
                nc.sync.dma_start(bt_sb[:], bt_in[b])
                rc_sb = spool.tile([128, NTOK], f32, tag="rc")
                nc.sync.dma_start(rc_sb[:], recip_in[b])

                # ---- phase B: gather + combine per token tile -----------
                for j in range(NTOK):
                    ge = gpool.tile([128, D], f32, tag="ge")
                    nc.gpsimd.indirect_dma_start(
                        out=ge[:], out_offset=None, in_=ptab[:],
                        in_offset=bass.IndirectOffsetOnAxis(
                            ap=idx_sb[:, 2 * j : 2 * j + 1], axis=0),
                    )
                    gs = gpool.tile([128, D], f32, tag="gs")
                    nc.gpsimd.indirect_dma_start(
                        out=gs[:], out_offset=None, in_=ptab[:],
                        in_offset=bass.IndirectOffsetOnAxis(
                            ap=idx_sb[:, 2 * j + 1 : 2 * j + 2], axis=0),
                    )
                    pstok = pstokpool.tile([128, D], f32, tag="pstok")
                    nc.tensor.matmul(out=pstok[:],
                                     lhsT=bt_sb[:, bass.ts(j, 128)],
                                     rhs=off_hi[:], start=True, stop=False)
                    nc.tensor.matmul(out=pstok[:],
                                     lhsT=bt_sb[:, bass.ts(j, 128)],
                                     rhs=off_lo[:], start=False, stop=True)
                    d1 = gpool.tile([128, D], f32, tag="d1")
                    nc.vector.tensor_sub(d1[:], ge[:], gs[:])
                    nc.vector.tensor_add(d1[:], d1[:], pstok[:])
                    ot = opool.tile([128, D], f32, tag="ot")
                    nc.scalar.mul(ot[:], d1[:], rc_sb[:, j : j + 1])
                    nc.sync.dma_start(seg_out[b, bass.ts(j, 128), :], ot[:])

    nc.compile()
    return nc


def _get_program():
    if "nc" not in _CACHE:
        _CACHE["nc"] = _build_program()
    return _CACHE["nc"]


def _host_prep(asr_alignment, text_token_len):
    a = np.asarray(asr_alignment).astype(np.int64)
    s, e = a[..., 0], a[..., 1]
    tlen = np.asarray(text_token_len).astype(np.int64)
    tmask = np.arange(TT)[None, :] < tlen[:, None]

    pe = np.where(((e + 1) % 128) != 0, e, ZROW)
    ps = np.where((s % 128) != 0, s - 1, ZROW)
    pe = np.where(tmask, pe, ZROW)
    ps = np.where(tmask, ps, ZROW)
    be = np.where(tmask, (e + 1) >> 7, 0)
    bs = np.where(tmask, s >> 7, 0)

    cnt = (e - s + 1).astype(np.float64)
    recip = np.where(tmask, 1.0 / cnt, 0.0).astype(np.float32)

    pidx = np.empty((B, 128, 2 * NTOK), np.int32)
    pidx[:, :, 0::2] = pe.reshape(B, NTOK, 128).transpose(0, 2, 1)
    pidx[:, :, 1::2] = ps.reshape(B, NTOK, 128).transpose(0, 2, 1)

    import ml_dtypes
    ks = np.arange(POFF)[None, :, None]
    bt = ((ks == be[:, None, :]).astype(np.float32)
          - (ks == bs[:, None, :]).astype(np.float32)).astype(ml_dtypes.bfloat16)

    recip_dev = recip.reshape(B, NTOK, 128).transpose(0, 2, 1)
    recip_dev = np.ascontiguousarray(recip_dev, np.float32)
    return pidx, np.ascontiguousarray(bt), recip_dev


def _run(inputs_by_core, trace=False, **kw):
    from concourse.bass_utils import run_bass_kernel_spmd
    nc = _get_program()
    return run_bass_kernel_spmd(nc, inputs_by_core,
                                core_ids=list(range(NCORES)), trace=trace, **kw)


def _make_in_maps(audio_feats, asr_alignment, text_token_len):
    import ml_dtypes
    audio = np.ascontiguousarray(np.asarray(audio_feats), np.float32)
    audio_hi = audio.astype(ml_dtypes.bfloat16)
    audio_lo = (audio - audio_hi.astype(np.float32)).astype(ml_dtypes.bfloat16)
    pidx, bt, recip = _host_prep(asr_alignment, text_token_len)
    lridx = (np.arange(NBLK, dtype=np.int32) * 128 + 127).reshape(NBLK, 1)
    in_maps = []
    for c in range(NCORES):
        sl = slice(BPC * c, BPC * (c + 1))
        in_maps.append({
            "audio_hi": audio_hi[sl],
            "audio_lo": audio_lo[sl],
            "pidx": pidx[sl],
            "bt": bt[sl],
            "recip": recip[sl],
            "lridx": lridx,
        })
    return in_maps


def kernel(audio_feats, audio_feats_len, text_token_for_audio,
           text_token_embed_for_audio, text_token_len, asr_alignment,
           _trace=False, **_kw):
    in_maps = _make_in_maps(audio_feats, asr_alignment, text_token_len)
    res = _run(in_maps, trace=_trace, **_kw)
    seg = np.concatenate([res.results[c]["seg"] for c in range(NCORES)], axis=0)
    out_len = np.asarray(text_token_len).astype(np.int32, copy=False)
    if _trace:
        return (seg, out_len), res
    return seg, out_len
